# revision 20
# baseline (speedup 1.0000x reference)
"""Bahdanau-style attention kernel for Trainium2, data-parallel over batch.

Math (per (s, b)):
    pre[s,b,:]  = We @ enc[s,b,:] + Wh @ hidden[b,:] + attn_b      (H outputs)
    energies    = score_w . tanh(pre)                               -> [S, B]
    out         = softmax over S of (energies masked)               -> [B, 1, S]

Sharding: B=16 batches split 2-per-core over 8 NeuronCores; weights are
replicated; no collectives.

v9: fp8(e4m3) DoubleRow main GEMM in the FLIPPED orientation (h_out on
PSUM partitions, We-pair stationary, enc-pair moving, K=256 per matmul).
  - We pre-scaled by 4096 on host (raw values are subnormal in e4m3);
    the 1/4096 un-scale + per-ho bias column (Wh@hidden_b + attn_b,
    f32 on host) fold into the ScalarE tanh (bias is per-partition in
    this orientation) - no DVE bias pass, no seed matmuls.
  - Score contraction: PE matmuls (score column stationary, proj
    moving), 4 s-blocks CONCURRENT via col-tiling tile_position=(0,32j)
    writing partition 32j of each s-block's just-freed main PSUM bank;
    DVE accumulates [1,512] slices into an SBUF energies row. Score
    groups are DEFERRED into the next window's kp-loop (PE MM queue is
    strict FIFO - a score MM waiting on tanh would stall the stream).
  - Mask folds into the first energies accumulation as an additive
    offset row (0 keep / -50 masked).
  - Loop: per batch, ho-pair windows x (kp outer, 4 sb inner) so each
    DoubleRow stationary amortizes over 4 matmuls. The LAST window
    runs pair-major (kp-chain per bank) so drains pipeline into the
    tail instead of bunching at the end.
  - Cold start: memsets + ~48 junk matmuls emitted before everything
    (HAM un-throttle by the time real matmuls stream) and tiny
    queue-warming DMAs ahead of the big prologue chunks.
  - Tail per batch: per-sb exp with fused accum_out, combine, DVE
    reciprocal, scale split ScalarE/DVE, one 8KB out DMA.
"""

import sys

for _p in ("/opt/trn_rl_repo", "/opt/pypackages"):
    if _p not in sys.path:
        sys.path.append(_p)

import numpy as np
import ml_dtypes

from concourse import bacc, mybir, tile
from concourse.bass_utils import run_bass_kernel_spmd

H = 1024
S = 2048
B = 16
NCORES = 8
BL = B // NCORES  # local batches per core
P = 128
KP = H // 256  # k-pairs of 256 = 4
HB = H // P  # ho blocks = 8
SB = S // 512  # s blocks of 512 = 4
WSCALE = 4096.0

F32 = mybir.dt.float32
BF16 = mybir.dt.bfloat16
FP8 = mybir.dt.float8e4
AF = mybir.ActivationFunctionType
OP = mybir.AluOpType
PM = mybir.MatmulPerfMode


def _build_program():
    nc = bacc.Bacc("TRN2", target_bir_lowering=False, debug=False, num_devices=NCORES)

    encP = nc.dram_tensor("encP", [BL, KP, P, 2 * S], FP8, kind="ExternalInput").ap()
    weP = nc.dram_tensor("weP", [KP, P, 2 * H], FP8, kind="ExternalInput").ap()
    biasc = nc.dram_tensor("biasc", [P, BL * HB], F32, kind="ExternalInput").ap()
    scorec = nc.dram_tensor("scorec", [P, HB], BF16, kind="ExternalInput").ap()
    moff = nc.dram_tensor("moff", [BL, S], F32, kind="ExternalInput").ap()
    out = nc.dram_tensor("out", [BL, S], F32, kind="ExternalOutput").ap()

    with tile.TileContext(nc) as tc:
        with (
            tc.tile_pool(name="consts", bufs=1) as cpool,
            tc.tile_pool(name="weights", bufs=1) as wpool,
            tc.tile_pool(name="enc", bufs=1) as epool,
            tc.tile_pool(name="work", bufs=14) as ppool,
            tc.tile_pool(name="soft", bufs=1) as spool,
            tc.tile_pool(name="mm", bufs=8, space="PSUM") as mmpool,
        ):
            # ---- warm-up first: memset consts + junk matmuls (one long
            # accumulation group so the stream is gapless - HAM needs a
            # full 3.4us continuously-busy window to un-throttle) --------
            ones_row_bf = cpool.tile([1, 512], BF16, tag="ones_row_bf")
            nc.vector.memset(ones_row_bf[:], 1.0)
            junk_ps = mmpool.tile([P, 512], F32, tag="mm", name="junk_ps")
            for _ in range(48):
                nc.tensor.matmul(
                    junk_ps[:, 0:P],
                    lhsT=ones_row_bf[:, 0:P],
                    rhs=ones_row_bf[:, 0:P],
                    start=True,
                    stop=True,
                    skip_group_check=True,
                )

            # ---- tiny constants first on each queue (doubles as queue
            # warming) ---------------------------------------------------
            bias_sb = cpool.tile([P, BL * HB], F32, tag="biasc", name="bias_sb")
            nc.sync.dma_start(bias_sb[:], biasc[:])
            score_sb = cpool.tile([P, HB], BF16, tag="scorec", name="score_sb")
            nc.gpsimd.dma_start(score_sb[:], scorec[:])
            moff_sb = []
            for b in range(BL):
                # single-partition rows go on SWDGE: an 8KB one-line DMA
                # ties up an HWDGE queue for ~3.4us
                m = cpool.tile([1, S], F32, tag=f"moff{b}", name=f"moff{b}")
                nc.gpsimd.dma_start(m[:], moff[b : b + 1, :])
                moff_sb.append(m)

            # ---- prologue: weP + enc-b0 halves spread over THREE DGE
            # queues (sync/scalar/vector) so batch 0's k-pairs land at
            # ~2-3us cadence -------------------------------------------
            we_sb = [
                wpool.tile([P, 2, H], FP8, tag=f"we{kp}", name=f"we{kp}")
                for kp in range(KP)
            ]
            enc_sb = [
                [
                    epool.tile([P, 2, S], FP8, tag=f"enc{b}_{kp}", name=f"enc{b}_{kp}")
                    for kp in range(KP)
                ]
                for b in range(BL)
            ]

            def enc_half(b, kp, h):
                dst = enc_sb[b][kp][:, :, h * 1024 : (h + 1) * 1024]
                src = encP[b, kp].rearrange("p (i s) -> p i s", i=2)[
                    :, :, h * 1024 : (h + 1) * 1024
                ]
                return dst, src

            def we_dma(q, kp):
                q.dma_start(we_sb[kp][:].rearrange("p i m -> p (i m)"), weP[kp])

            def eh_dma(q, b, kp, h):
                d_, s_ = enc_half(b, kp, h)
                q.dma_start(d_, s_)

            # per-queue chunk order (256KB each); HWDGE queues only -
            # big SWDGE transfers stall (software descriptor generation)
            we_dma(nc.sync, 0)
            eh_dma(nc.sync, 0, 1, 1)
            eh_dma(nc.sync, 0, 2, 0)
            we_dma(nc.sync, 2)
            eh_dma(nc.sync, 0, 3, 0)
            eh_dma(nc.sync, 0, 3, 1)
            eh_dma(nc.scalar, 0, 0, 0)
            eh_dma(nc.scalar, 0, 0, 1)
            we_dma(nc.scalar, 1)
            eh_dma(nc.scalar, 0, 1, 0)
            eh_dma(nc.scalar, 0, 2, 1)
            we_dma(nc.scalar, 3)

            # batch 1: full-chunk DMAs (2KB/partition lines)
            for kp, q in zip(range(KP), (nc.sync, nc.scalar, nc.sync, nc.scalar)):
                q.dma_start(
                    enc_sb[1][kp][:].rearrange("p i s -> p (i s)"), encP[1, kp]
                )

            en_acc = [
                spool.tile([1, S], F32, tag=f"en_acc{b}", name=f"en_acc{b}")
                for b in range(BL)
            ]
            exp_sb = [
                spool.tile([1, S], F32, tag=f"expd{b}", name=f"expd{b}")
                for b in range(BL)
            ]
            tot_sb = [
                spool.tile([1, SB], F32, tag=f"tot{b}", name=f"tot{b}")
                for b in range(BL)
            ]

            def drains(b, ho, ps_row, last, seed=False):
                """tanh all 4 s-blocks of one ho, then a col-tiled score
                matmul 4-pack + DVE energy accumulation.

                Returns a closure emitting the score pack + DVE adds; in
                steady state the caller defers it into the next window's
                kp-loop so PE-FIFO stalls on tanh never block DR matmuls.
                """
                bcol = bias_sb[:, b * HB + ho : b * HB + ho + 1]
                projs = []
                for sb in range(SB):
                    proj = ppool.tile(
                        [P, 512], BF16, tag="proj", name=f"proj{b}_{ho}_{sb}"
                    )
                    nc.scalar.activation(
                        proj[:], ps_row[sb][:], AF.Tanh, bias=bcol, scale=1.0 / WSCALE
                    )
                    projs.append(proj)

                def emit_scores():
                    for sb in range(SB):
                        en_ps = ps_row[sb][32 * sb : 32 * sb + 1, :]
                        nc.tensor.matmul(
                            en_ps,
                            lhsT=score_sb[:, ho : ho + 1],
                            rhs=projs[sb][:],
                            start=True,
                            stop=True,
                            tile_position=(0, 32 * sb),
                        )
                    for sb in range(SB):
                        en_ps = ps_row[sb][32 * sb : 32 * sb + 1, :]
                        acc = en_acc[b][:, sb * 512 : (sb + 1) * 512]
                        if seed:
                            nc.vector.scalar_tensor_tensor(
                                acc,
                                en_ps,
                                0.0,
                                moff_sb[b][:, sb * 512 : (sb + 1) * 512],
                                op0=OP.bypass,
                                op1=OP.add,
                            )
                        else:
                            nc.vector.tensor_tensor(acc, acc, en_ps, op=OP.add)
                        if last:
                            # pipeline the tail: exp + per-sb total
                            nc.scalar.activation(
                                exp_sb[b][:, sb * 512 : (sb + 1) * 512],
                                acc,
                                AF.Exp,
                                accum_out=tot_sb[b][:, sb : sb + 1],
                            )

                return emit_scores

            def tail_combine(b):
                """Combine per-sb exp totals, reciprocal, scale, store."""
                t01 = spool.tile([1, 1], F32, tag=f"t01_{b}", name=f"t01_{b}")
                t23 = spool.tile([1, 1], F32, tag=f"t23_{b}", name=f"t23_{b}")
                tot = spool.tile([1, 1], F32, tag=f"tt_{b}", name=f"tt_{b}")
                nc.vector.tensor_tensor(
                    t01[:], tot_sb[b][:, 0:1], tot_sb[b][:, 1:2], op=OP.add
                )
                nc.vector.tensor_tensor(
                    t23[:], tot_sb[b][:, 2:3], tot_sb[b][:, 3:4], op=OP.add
                )
                nc.vector.tensor_tensor(tot[:], t01[:], t23[:], op=OP.add)
                rec = spool.tile([1, 1], F32, tag=f"rec{b}", name=f"rec{b}")
                nc.vector.reciprocal(rec[:], tot[:])
                outrow = spool.tile([1, S], F32, tag=f"outrow{b}", name=f"outrow{b}")
                for q in range(SB):
                    sl = slice(q * 512, (q + 1) * 512)
                    if q == 0:
                        nc.scalar.mul(outrow[:, sl], exp_sb[b][:, sl], rec[:])
                    else:
                        nc.vector.tensor_scalar(
                            outrow[:, sl], exp_sb[b][:, sl], rec[:], None, op0=OP.mult
                        )
                    dq = nc.sync if q % 2 == 0 else nc.scalar
                    dq.dma_start(out[b : b + 1, sl], outrow[:, sl])

            def tail_full(b):
                """Tail for a batch whose drains didn't pipeline exp."""
                for sb in range(SB):
                    sl = slice(sb * 512, (sb + 1) * 512)
                    nc.scalar.activation(
                        exp_sb[b][:, sl],
                        en_acc[b][:, sl],
                        AF.Exp,
                        accum_out=tot_sb[b][:, sb : sb + 1],
                    )
                tail_combine(b)

            # ---- main loops ---------------------------------------------
            # Phase A: b0 ho0-1 pair window (prologue: only b0+weP needed)
            # Phase B: ho2..7 merged across BOTH batches - single-ho
            #          windows where each (kp,ho) stationary feeds 8
            #          matmuls (LDWEIGHTS amortized 8x)
            # Phase C: b1 ho0-1 pair window + pipelined tail
            pending = []

            def steady_pair_window(b, W, seed_b, last_b=False):
                nonlocal pending
                ps = [
                    [
                        mmpool.tile(
                            [P, 512], F32, tag="mm", name=f"ps{b}_{W}_{hi}_{sb}"
                        )
                        for sb in range(SB)
                    ]
                    for hi in range(2)
                ]
                for kp in range(KP):
                    for hi in range(2):
                        ho = 2 * W + hi
                        wsl = we_sb[kp][:, :, ho * P : (ho + 1) * P]
                        for sb in range(SB):
                            nc.tensor.matmul(
                                ps[hi][sb][:],
                                lhsT=wsl,
                                rhs=enc_sb[b][kp][:, :, sb * 512 : (sb + 1) * 512],
                                start=(kp == 0),
                                stop=(kp == KP - 1),
                                perf_mode=PM.DoubleRow,
                            )
                    if kp == 1:
                        for fn in pending:
                            fn()
                        pending = []
                for hi in range(2):
                    ho = 2 * W + hi
                    em = drains(
                        b, ho, ps[hi], last=(last_b and hi == 1),
                        seed=(seed_b and W == 0 and hi == 0),
                    )
                    if hi == 0 or last_b:
                        em()
                    else:
                        pending.append(em)
                if last_b:
                    tail_combine(b)

            steady_pair_window(0, 0, seed_b=True)

            for ho in range(2, HB):
                ps = [
                    [
                        mmpool.tile(
                            [P, 512], F32, tag="mm", name=f"psM{b}_{ho}_{sb}"
                        )
                        for sb in range(SB)
                    ]
                    for b in range(BL)
                ]
                for kp in range(KP):
                    wsl = we_sb[kp][:, :, ho * P : (ho + 1) * P]
                    for b in range(BL):
                        for sb in range(SB):
                            nc.tensor.matmul(
                                ps[b][sb][:],
                                lhsT=wsl,
                                rhs=enc_sb[b][kp][:, :, sb * 512 : (sb + 1) * 512],
                                start=(kp == 0),
                                stop=(kp == KP - 1),
                                perf_mode=PM.DoubleRow,
                            )
                    if kp == 1:
                        for fn in pending:
                            fn()
                        pending = []
                for b in range(BL):
                    em = drains(b, ho, ps[b], last=False, seed=(b == 1 and ho == 2))
                    if b == 0:
                        em()  # b0 banks finish ~1us before window end
                    else:
                        pending.append(em)

            # Phase C: b1's ho0-1; flush leftovers + b0's tail inside
            ps = [
                [
                    mmpool.tile([P, 512], F32, tag="mm", name=f"psC_{hi}_{sb}")
                    for sb in range(SB)
                ]
                for hi in range(2)
            ]
            first = True
            for hi in range(2):
                ho = hi
                for sb in range(SB):
                    for kp in range(KP):
                        nc.tensor.matmul(
                            ps[hi][sb][:],
                            lhsT=we_sb[kp][:, :, ho * P : (ho + 1) * P],
                            rhs=enc_sb[1][kp][:, :, sb * 512 : (sb + 1) * 512],
                            start=(kp == 0),
                            stop=(kp == KP - 1),
                            perf_mode=PM.DoubleRow,
                        )
                    if first:
                        for fn in pending:
                            fn()
                        pending = []
                        tail_full(0)
                        first = False
                em = drains(1, ho, ps[hi], last=(hi == 1), seed=False)
                em()
            tail_combine(1)

    nc.compile()
    return nc


_NC = None


def _get_program():
    global _NC
    if _NC is None:
        _NC = _build_program()
    return _NC


def make_in_maps(hidden, encoder_outputs, seq_mask, attn_w, attn_b, score_w):
    """Slice/relayout/quantize the full inputs into 8 per-core input maps."""
    hidden = np.asarray(hidden, dtype=np.float32)
    encoder_outputs = np.asarray(encoder_outputs, dtype=np.float32)
    seq_mask = np.asarray(seq_mask, dtype=np.int32)
    attn_w = np.asarray(attn_w, dtype=np.float32)
    attn_b = np.asarray(attn_b, dtype=np.float32)
    score_w = np.asarray(score_w, dtype=np.float32)

    e4 = ml_dtypes.float8_e4m3fn
    bf = ml_dtypes.bfloat16

    # We^T [hin, ho] scaled into e4m3's normal range; pair layout
    # [KP, P, 2, ho]: hin = kp*256 + i*128 + p
    weT = attn_w[:, H:].T * WSCALE
    weP = np.ascontiguousarray(
        weT.reshape(KP, 2, P, H).transpose(0, 2, 1, 3).reshape(KP, P, 2 * H)
    ).astype(e4)

    # enc pair layout per batch: [B, KP, P, 2, S]
    encT = encoder_outputs.transpose(1, 2, 0)  # [B, H, S]
    encPf = encT.reshape(B, KP, 2, P, S).transpose(0, 1, 3, 2, 4).reshape(
        B, KP, P, 2 * S
    )
    encPq = np.ascontiguousarray(encPf).astype(e4)

    # bias columns: Wh @ hidden_b + attn_b, exact f32, [P, B*HB] with
    # col = b*HB + hb, row p = h_out hb*128+p
    hidb = hidden[0] @ attn_w[:, :H].T + attn_b  # [B, H]
    biasc_all = hidb.reshape(B, HB, P).transpose(2, 0, 1).reshape(P, B * HB)
    biasc_all = np.ascontiguousarray(biasc_all.astype(np.float32))

    scorec = np.ascontiguousarray(score_w[0].reshape(HB, P).T).astype(bf)

    # additive mask offsets: 0 keep, -50 masked (exp -> ~1e-22)
    moff_all = np.where(seq_mask != 0, np.float32(-50.0), np.float32(0.0))

    in_maps = []
    for c in range(NCORES):
        bsl = slice(c * BL, (c + 1) * BL)
        in_maps.append(
            {
                "encP": np.ascontiguousarray(encPq[bsl]),
                "weP": weP,
                "biasc": np.ascontiguousarray(
                    biasc_all[:, c * BL * HB : (c + 1) * BL * HB]
                ),
                "scorec": scorec,
                "moff": np.ascontiguousarray(moff_all[bsl]),
            }
        )
    return in_maps


def gather_output(results):
    outs = np.concatenate([results[c]["out"] for c in range(NCORES)], axis=0)
    return np.ascontiguousarray(outs[:, None, :].astype(np.float32))


def kernel(hidden, encoder_outputs, seq_mask, attn_w, attn_b, score_w):
    nc = _get_program()
    in_maps = make_in_maps(
        hidden, encoder_outputs, seq_mask, attn_w, attn_b, score_w
    )
    last_err = None
    for _attempt in range(3):
        try:
            res = run_bass_kernel_spmd(nc, in_maps, list(range(NCORES)))
            return gather_output(res.results)
        except Exception as e:  # rare transient NRT device errors on first exec
            last_err = e
            import time as _time

            _time.sleep(2.0)
    raise last_err


# revision 21
# speedup vs baseline: 1.0326x; 1.0326x over previous
"""Bahdanau-style attention kernel for Trainium2, data-parallel over batch.

Math (per (s, b)):
    pre[s,b,:]  = We @ enc[s,b,:] + Wh @ hidden[b,:] + attn_b      (H outputs)
    energies    = score_w . tanh(pre)                               -> [S, B]
    out         = softmax over S of (energies masked)               -> [B, 1, S]

Sharding: B=16 batches split 2-per-core over 8 NeuronCores; weights are
replicated; no collectives.

v9: fp8(e4m3) DoubleRow main GEMM in the FLIPPED orientation (h_out on
PSUM partitions, We-pair stationary, enc-pair moving, K=256 per matmul).
  - We pre-scaled by 4096 on host (raw values are subnormal in e4m3);
    the 1/4096 un-scale + per-ho bias column (Wh@hidden_b + attn_b,
    f32 on host) fold into the ScalarE tanh (bias is per-partition in
    this orientation) - no DVE bias pass, no seed matmuls.
  - Score contraction: PE matmuls (score column stationary, proj
    moving), 4 s-blocks CONCURRENT via col-tiling tile_position=(0,32j)
    writing partition 32j of each s-block's just-freed main PSUM bank;
    DVE accumulates [1,512] slices into an SBUF energies row. Score
    groups are DEFERRED into the next window's kp-loop (PE MM queue is
    strict FIFO - a score MM waiting on tanh would stall the stream).
  - Mask folds into the first energies accumulation as an additive
    offset row (0 keep / -50 masked).
  - Loop: per batch, ho-pair windows x (kp outer, 4 sb inner) so each
    DoubleRow stationary amortizes over 4 matmuls. The LAST window
    runs pair-major (kp-chain per bank) so drains pipeline into the
    tail instead of bunching at the end.
  - Cold start: memsets + ~48 junk matmuls emitted before everything
    (HAM un-throttle by the time real matmuls stream) and tiny
    queue-warming DMAs ahead of the big prologue chunks.
  - Tail per batch: per-sb exp with fused accum_out, combine, DVE
    reciprocal, scale split ScalarE/DVE, one 8KB out DMA.
"""

import sys

for _p in ("/opt/trn_rl_repo", "/opt/pypackages"):
    if _p not in sys.path:
        sys.path.append(_p)

import numpy as np
import ml_dtypes

from concourse import bacc, mybir, tile
from concourse.bass_utils import run_bass_kernel_spmd

H = 1024
S = 2048
B = 16
NCORES = 8
BL = B // NCORES  # local batches per core
P = 128
KP = H // 256  # k-pairs of 256 = 4
HB = H // P  # ho blocks = 8
SB = S // 512  # s blocks of 512 = 4
WSCALE = 4096.0

F32 = mybir.dt.float32
BF16 = mybir.dt.bfloat16
FP8 = mybir.dt.float8e4
AF = mybir.ActivationFunctionType
OP = mybir.AluOpType
PM = mybir.MatmulPerfMode


def _build_program():
    nc = bacc.Bacc("TRN2", target_bir_lowering=False, debug=False, num_devices=NCORES)

    encP = nc.dram_tensor("encP", [BL, KP, P, 2 * S], FP8, kind="ExternalInput").ap()
    weP = nc.dram_tensor("weP", [KP, P, 2 * H], FP8, kind="ExternalInput").ap()
    biasc = nc.dram_tensor("biasc", [P, BL * HB], F32, kind="ExternalInput").ap()
    scorec = nc.dram_tensor("scorec", [P, HB], BF16, kind="ExternalInput").ap()
    moff = nc.dram_tensor("moff", [BL, S], F32, kind="ExternalInput").ap()
    out = nc.dram_tensor("out", [BL, S], F32, kind="ExternalOutput").ap()

    with tile.TileContext(nc) as tc:
        with (
            tc.tile_pool(name="consts", bufs=1) as cpool,
            tc.tile_pool(name="weights", bufs=1) as wpool,
            tc.tile_pool(name="enc", bufs=1) as epool,
            tc.tile_pool(name="work", bufs=14) as ppool,
            tc.tile_pool(name="soft", bufs=1) as spool,
            tc.tile_pool(name="mm", bufs=8, space="PSUM") as mmpool,
        ):
            # ---- warm-up first: memset consts + junk matmuls (one long
            # accumulation group so the stream is gapless - HAM needs a
            # full 3.4us continuously-busy window to un-throttle) --------
            ones_row_bf = cpool.tile([1, 512], BF16, tag="ones_row_bf")
            nc.vector.memset(ones_row_bf[:], 1.0)
            junk_ps = mmpool.tile([P, 512], F32, tag="mm", name="junk_ps")
            for _ in range(48):
                nc.tensor.matmul(
                    junk_ps[:, 0:P],
                    lhsT=ones_row_bf[:, 0:P],
                    rhs=ones_row_bf[:, 0:P],
                    start=True,
                    stop=True,
                    skip_group_check=True,
                )

            # ---- tiny constants first on each queue (doubles as queue
            # warming) ---------------------------------------------------
            bias_sb = cpool.tile([P, BL * HB], F32, tag="biasc", name="bias_sb")
            nc.sync.dma_start(bias_sb[:], biasc[:])
            score_sb = cpool.tile([P, HB], BF16, tag="scorec", name="score_sb")
            nc.gpsimd.dma_start(score_sb[:], scorec[:])
            moff_sb = []
            for b in range(BL):
                # single-partition rows go on SWDGE: an 8KB one-line DMA
                # ties up an HWDGE queue for ~3.4us
                m = cpool.tile([1, S], F32, tag=f"moff{b}", name=f"moff{b}")
                nc.gpsimd.dma_start(m[:], moff[b : b + 1, :])
                moff_sb.append(m)

            # ---- prologue: weP + enc-b0 halves spread over THREE DGE
            # queues (sync/scalar/vector) so batch 0's k-pairs land at
            # ~2-3us cadence -------------------------------------------
            we_sb = [
                wpool.tile([P, 2, H], FP8, tag=f"we{kp}", name=f"we{kp}")
                for kp in range(KP)
            ]
            enc_sb = [
                [
                    epool.tile([P, 2, S], FP8, tag=f"enc{b}_{kp}", name=f"enc{b}_{kp}")
                    for kp in range(KP)
                ]
                for b in range(BL)
            ]

            def enc_half(b, kp, h):
                dst = enc_sb[b][kp][:, :, h * 1024 : (h + 1) * 1024]
                src = encP[b, kp].rearrange("p (i s) -> p i s", i=2)[
                    :, :, h * 1024 : (h + 1) * 1024
                ]
                return dst, src

            def we_dma(q, kp):
                q.dma_start(we_sb[kp][:].rearrange("p i m -> p (i m)"), weP[kp])

            def eh_dma(q, b, kp, h):
                d_, s_ = enc_half(b, kp, h)
                q.dma_start(d_, s_)

            # per-queue chunk order (256KB each); HWDGE queues only -
            # big SWDGE transfers stall (software descriptor generation)
            we_dma(nc.sync, 0)
            eh_dma(nc.sync, 0, 1, 1)
            eh_dma(nc.sync, 0, 2, 0)
            we_dma(nc.sync, 2)
            eh_dma(nc.sync, 0, 3, 0)
            eh_dma(nc.sync, 0, 3, 1)
            eh_dma(nc.scalar, 0, 0, 0)
            eh_dma(nc.scalar, 0, 0, 1)
            we_dma(nc.scalar, 1)
            eh_dma(nc.scalar, 0, 1, 0)
            eh_dma(nc.scalar, 0, 2, 1)
            we_dma(nc.scalar, 3)

            # batch 1: full-chunk DMAs (2KB/partition lines)
            for kp, q in zip(range(KP), (nc.sync, nc.scalar, nc.sync, nc.scalar)):
                q.dma_start(
                    enc_sb[1][kp][:].rearrange("p i s -> p (i s)"), encP[1, kp]
                )

            en_acc = [
                spool.tile([1, S], F32, tag=f"en_acc{b}", name=f"en_acc{b}")
                for b in range(BL)
            ]
            exp_sb = [
                spool.tile([1, S], F32, tag=f"expd{b}", name=f"expd{b}")
                for b in range(BL)
            ]
            tot_sb = [
                spool.tile([1, SB], F32, tag=f"tot{b}", name=f"tot{b}")
                for b in range(BL)
            ]

            def drains(b, ho, ps_row, last, seed=False):
                """tanh all 4 s-blocks of one ho, then a col-tiled score
                matmul 4-pack + DVE energy accumulation.

                Returns a closure emitting the score pack + DVE adds; in
                steady state the caller defers it into the next window's
                kp-loop so PE-FIFO stalls on tanh never block DR matmuls.
                """
                bcol = bias_sb[:, b * HB + ho : b * HB + ho + 1]
                projs = []
                for sb in range(SB):
                    proj = ppool.tile(
                        [P, 512], BF16, tag="proj", name=f"proj{b}_{ho}_{sb}"
                    )
                    nc.scalar.activation(
                        proj[:], ps_row[sb][:], AF.Tanh, bias=bcol, scale=1.0 / WSCALE
                    )
                    projs.append(proj)

                def emit_scores():
                    for sb in range(SB):
                        en_ps = ps_row[sb][32 * sb : 32 * sb + 1, :]
                        nc.tensor.matmul(
                            en_ps,
                            lhsT=score_sb[:, ho : ho + 1],
                            rhs=projs[sb][:],
                            start=True,
                            stop=True,
                            tile_position=(0, 32 * sb),
                        )
                    for sb in range(SB):
                        en_ps = ps_row[sb][32 * sb : 32 * sb + 1, :]
                        acc = en_acc[b][:, sb * 512 : (sb + 1) * 512]
                        if seed:
                            nc.vector.scalar_tensor_tensor(
                                acc,
                                en_ps,
                                0.0,
                                moff_sb[b][:, sb * 512 : (sb + 1) * 512],
                                op0=OP.bypass,
                                op1=OP.add,
                            )
                        else:
                            nc.vector.tensor_tensor(acc, acc, en_ps, op=OP.add)
                        if last:
                            # pipeline the tail: exp + per-sb total
                            nc.scalar.activation(
                                exp_sb[b][:, sb * 512 : (sb + 1) * 512],
                                acc,
                                AF.Exp,
                                accum_out=tot_sb[b][:, sb : sb + 1],
                            )

                return emit_scores

            def tail_combine(b):
                """Combine per-sb exp totals, reciprocal, scale, store."""
                t01 = spool.tile([1, 1], F32, tag=f"t01_{b}", name=f"t01_{b}")
                t23 = spool.tile([1, 1], F32, tag=f"t23_{b}", name=f"t23_{b}")
                tot = spool.tile([1, 1], F32, tag=f"tt_{b}", name=f"tt_{b}")
                nc.vector.tensor_tensor(
                    t01[:], tot_sb[b][:, 0:1], tot_sb[b][:, 1:2], op=OP.add
                )
                nc.vector.tensor_tensor(
                    t23[:], tot_sb[b][:, 2:3], tot_sb[b][:, 3:4], op=OP.add
                )
                nc.vector.tensor_tensor(tot[:], t01[:], t23[:], op=OP.add)
                rec = spool.tile([1, 1], F32, tag=f"rec{b}", name=f"rec{b}")
                nc.vector.reciprocal(rec[:], tot[:])
                outrow = spool.tile([1, S], F32, tag=f"outrow{b}", name=f"outrow{b}")
                for q in range(SB):
                    sl = slice(q * 512, (q + 1) * 512)
                    if q == 0:
                        nc.scalar.mul(outrow[:, sl], exp_sb[b][:, sl], rec[:])
                    else:
                        nc.vector.tensor_scalar(
                            outrow[:, sl], exp_sb[b][:, sl], rec[:], None, op0=OP.mult
                        )
                    dq = nc.sync if q % 2 == 0 else nc.scalar
                    dq.dma_start(out[b : b + 1, sl], outrow[:, sl])

            def tail_full(b):
                """Tail for a batch whose drains didn't pipeline exp."""
                for sb in range(SB):
                    sl = slice(sb * 512, (sb + 1) * 512)
                    nc.scalar.activation(
                        exp_sb[b][:, sl],
                        en_acc[b][:, sl],
                        AF.Exp,
                        accum_out=tot_sb[b][:, sb : sb + 1],
                    )
                tail_combine(b)

            # ---- main loops ---------------------------------------------
            # Phase A: b0 ho0-1 pair window (prologue: only b0+weP needed)
            # Phase B: ho2..7 merged across BOTH batches - single-ho
            #          windows where each (kp,ho) stationary feeds 8
            #          matmuls (LDWEIGHTS amortized 8x)
            # Phase C: b1 ho0-1 pair window + pipelined tail
            pending = []

            def steady_pair_window(b, W, seed_b, last_b=False):
                nonlocal pending
                ps = [
                    [
                        mmpool.tile(
                            [P, 512], F32, tag="mm", name=f"ps{b}_{W}_{hi}_{sb}"
                        )
                        for sb in range(SB)
                    ]
                    for hi in range(2)
                ]
                for kp in range(KP):
                    for hi in range(2):
                        ho = 2 * W + hi
                        wsl = we_sb[kp][:, :, ho * P : (ho + 1) * P]
                        for sb in range(SB):
                            nc.tensor.matmul(
                                ps[hi][sb][:],
                                lhsT=wsl,
                                rhs=enc_sb[b][kp][:, :, sb * 512 : (sb + 1) * 512],
                                start=(kp == 0),
                                stop=(kp == KP - 1),
                                perf_mode=PM.DoubleRow,
                            )
                    if kp == 1:
                        for fn in pending:
                            fn()
                        pending = []
                for hi in range(2):
                    ho = 2 * W + hi
                    em = drains(
                        b, ho, ps[hi], last=(last_b and hi == 1),
                        seed=(seed_b and W == 0 and hi == 0),
                    )
                    if hi == 0 or last_b:
                        em()
                    else:
                        pending.append(em)
                if last_b:
                    tail_combine(b)

            steady_pair_window(0, 0, seed_b=True)

            for ho in range(2, HB):
                ps = [
                    [
                        mmpool.tile(
                            [P, 512], F32, tag="mm", name=f"psM{b}_{ho}_{sb}"
                        )
                        for sb in range(SB)
                    ]
                    for b in range(BL)
                ]
                for kp in range(KP):
                    wsl = we_sb[kp][:, :, ho * P : (ho + 1) * P]
                    for b in range(BL):
                        for sb in range(SB):
                            nc.tensor.matmul(
                                ps[b][sb][:],
                                lhsT=wsl,
                                rhs=enc_sb[b][kp][:, :, sb * 512 : (sb + 1) * 512],
                                start=(kp == 0),
                                stop=(kp == KP - 1),
                                perf_mode=PM.DoubleRow,
                            )
                    if kp == 1:
                        for fn in pending:
                            fn()
                        pending = []
                for b in range(BL):
                    em = drains(b, ho, ps[b], last=False, seed=(b == 1 and ho == 2))
                    if b == 0:
                        em()  # b0 banks finish ~1us before window end
                    else:
                        pending.append(em)

            # Phase C: b1's ho0-1; flush leftovers + b0's tail inside
            ps = [
                [
                    mmpool.tile([P, 512], F32, tag="mm", name=f"psC_{hi}_{sb}")
                    for sb in range(SB)
                ]
                for hi in range(2)
            ]
            first = True
            for hi in range(2):
                ho = hi
                for sb in range(SB):
                    for kp in range(KP):
                        nc.tensor.matmul(
                            ps[hi][sb][:],
                            lhsT=we_sb[kp][:, :, ho * P : (ho + 1) * P],
                            rhs=enc_sb[1][kp][:, :, sb * 512 : (sb + 1) * 512],
                            start=(kp == 0),
                            stop=(kp == KP - 1),
                            perf_mode=PM.DoubleRow,
                        )
                    if first:
                        for fn in pending:
                            fn()
                        pending = []
                        tail_full(0)
                        first = False
                # both ho's score packs ACCUMULATE into the hi=0 banks'
                # [32sb] slices (last window - banks never reused), so
                # only 4 DVE adds + 4 exps remain on the exposed tail
                bcol = bias_sb[:, 1 * HB + ho : 1 * HB + ho + 1]
                projs = []
                for sb in range(SB):
                    proj = ppool.tile(
                        [P, 512], BF16, tag="proj", name=f"projC{ho}_{sb}"
                    )
                    nc.scalar.activation(
                        proj[:], ps[hi][sb][:], AF.Tanh, bias=bcol, scale=1.0 / WSCALE
                    )
                    projs.append(proj)
                for sb in range(SB):
                    nc.tensor.matmul(
                        ps[0][sb][32 * sb : 32 * sb + 1, :],
                        lhsT=score_sb[:, ho : ho + 1],
                        rhs=projs[sb][:],
                        start=(hi == 0),
                        stop=(hi == 1),
                        tile_position=(0, 32 * sb),
                    )
            for sb in range(SB):
                en_ps = ps[0][sb][32 * sb : 32 * sb + 1, :]
                acc = en_acc[1][:, sb * 512 : (sb + 1) * 512]
                nc.vector.tensor_tensor(acc, acc, en_ps, op=OP.add)
                nc.scalar.activation(
                    exp_sb[1][:, sb * 512 : (sb + 1) * 512],
                    acc,
                    AF.Exp,
                    accum_out=tot_sb[1][:, sb : sb + 1],
                )
            tail_combine(1)

    nc.compile()
    return nc


_NC = None


def _get_program():
    global _NC
    if _NC is None:
        _NC = _build_program()
    return _NC


def make_in_maps(hidden, encoder_outputs, seq_mask, attn_w, attn_b, score_w):
    """Slice/relayout/quantize the full inputs into 8 per-core input maps."""
    hidden = np.asarray(hidden, dtype=np.float32)
    encoder_outputs = np.asarray(encoder_outputs, dtype=np.float32)
    seq_mask = np.asarray(seq_mask, dtype=np.int32)
    attn_w = np.asarray(attn_w, dtype=np.float32)
    attn_b = np.asarray(attn_b, dtype=np.float32)
    score_w = np.asarray(score_w, dtype=np.float32)

    e4 = ml_dtypes.float8_e4m3fn
    bf = ml_dtypes.bfloat16

    # We^T [hin, ho] scaled into e4m3's normal range; pair layout
    # [KP, P, 2, ho]: hin = kp*256 + i*128 + p
    weT = attn_w[:, H:].T * WSCALE
    weP = np.ascontiguousarray(
        weT.reshape(KP, 2, P, H).transpose(0, 2, 1, 3).reshape(KP, P, 2 * H)
    ).astype(e4)

    # enc pair layout per batch: [B, KP, P, 2, S]
    encT = encoder_outputs.transpose(1, 2, 0)  # [B, H, S]
    encPf = encT.reshape(B, KP, 2, P, S).transpose(0, 1, 3, 2, 4).reshape(
        B, KP, P, 2 * S
    )
    encPq = np.ascontiguousarray(encPf).astype(e4)

    # bias columns: Wh @ hidden_b + attn_b, exact f32, [P, B*HB] with
    # col = b*HB + hb, row p = h_out hb*128+p
    hidb = hidden[0] @ attn_w[:, :H].T + attn_b  # [B, H]
    biasc_all = hidb.reshape(B, HB, P).transpose(2, 0, 1).reshape(P, B * HB)
    biasc_all = np.ascontiguousarray(biasc_all.astype(np.float32))

    scorec = np.ascontiguousarray(score_w[0].reshape(HB, P).T).astype(bf)

    # additive mask offsets: 0 keep, -50 masked (exp -> ~1e-22)
    moff_all = np.where(seq_mask != 0, np.float32(-50.0), np.float32(0.0))

    in_maps = []
    for c in range(NCORES):
        bsl = slice(c * BL, (c + 1) * BL)
        in_maps.append(
            {
                "encP": np.ascontiguousarray(encPq[bsl]),
                "weP": weP,
                "biasc": np.ascontiguousarray(
                    biasc_all[:, c * BL * HB : (c + 1) * BL * HB]
                ),
                "scorec": scorec,
                "moff": np.ascontiguousarray(moff_all[bsl]),
            }
        )
    return in_maps


def gather_output(results):
    outs = np.concatenate([results[c]["out"] for c in range(NCORES)], axis=0)
    return np.ascontiguousarray(outs[:, None, :].astype(np.float32))


def kernel(hidden, encoder_outputs, seq_mask, attn_w, attn_b, score_w):
    nc = _get_program()
    in_maps = make_in_maps(
        hidden, encoder_outputs, seq_mask, attn_w, attn_b, score_w
    )
    last_err = None
    for _attempt in range(3):
        try:
            res = run_bass_kernel_spmd(nc, in_maps, list(range(NCORES)))
            return gather_output(res.results)
        except Exception as e:  # rare transient NRT device errors on first exec
            last_err = e
            import time as _time

            _time.sleep(2.0)
    raise last_err
